# revision 12
# baseline (speedup 1.0000x reference)
"""CredLightGCN (3-layer LightGCN propagation + batch dot readout) on 8
Trainium2 NeuronCores.

Strategy (all sizes hardcoded for the nn_CredLightGCN problem):
  * The six SpMMs (2 directions x 3 layers) are computed as PE one-hot
    matmuls: for each destination group of 128 rows, PSUM accumulates
    chunks  out[seg,d] += OH[slot,seg]^T @ G[slot,d]  where OH is a
    one-hot (edge-value-weighted) selection matrix generated ON DEVICE by
    the vector engines:  OH[p, f] = (iota[f] == seg[p]) * val[p]  via a
    fused tensor_scalar(is_equal, mult) with per-partition scalars.  Only
    the tiny seg/val streams (4B/edge) come from HBM, not the 32KB/chunk
    dense M tiles.  One-hot generation alternates DVE / Pool to balance
    engine load.  One slot per edge (no dedup).
  * Layer 1 needs no on-device gathers: G streams from host-expanded edge
    tables (the inputs are known on the host).
  * Layer 2 gathers source rows with gpsimd dma_gather (256B rows, int16
    indices, tables split in 25088-row windows).
  * Layer 3 is batch-funneled: only rows reachable from the 16384 query
    pairs are produced (batch positions are the destination rows).
  * Tables are bf16 padded to 128 cols (256B rows) to satisfy dma_gather's
    256B element constraint; PSUM accumulation stays f32.
  * Cores own disjoint destination-row shards; full tables are rebuilt
    between passes with DRAM AllGather collectives.
    Pass order (ui-L1, iu-L1, iu-L2, ui-L2, l3a, l3b) is chosen
    so every AllGather overlaps the pass that follows it; the next
    gather-pass's first gathers carry explicit deps on the previous
    pass's Pool tail so a gather parked on its AllGather wait cannot
    head-of-line block Pool.SEQ.
  * Readout: gather s1/s2 rows as 1KB "quad" rows (4 padded rows per
    descriptor, index = row//4) in ONE dma_gather per (side, table),
    select the right sub-row with a bf16 mask + axis reduce, add the
    layer-0 and layer-3 terms, multiply sides and row-reduce.

Row permutation: items/users are assigned to device rows by a
degree-balanced snake so every destination group has a near-equal edge
count, which makes the chunk schedule uniform across the 8 cores (all
cores run one shared program; per-core data differs).
"""

import numpy as np
import ml_dtypes

NCORES = 8
GSZ = 128         # dst rows per group (PSUM partitions)
CH = 128          # edge slots per chunk (PE contraction K)
D = 64            # embedding dim
DP = 128          # padded bf16 row width (256B)
BF = ml_dtypes.bfloat16

N_IT_REAL, N_US_REAL = 50000, 100000
UNIT = NCORES * GSZ
N_IT = -(-N_IT_REAL // UNIT) * UNIT          # 50176
N_US = -(-N_US_REAL // UNIT) * UNIT          # 100352
GI, GU = N_IT // GSZ, N_US // GSZ
GI_C, GU_C = GI // NCORES, GU // NCORES
QS = 25088
NQ_US, NQ_IT = -(-N_US // QS), -(-N_IT // QS)    # 4, 2
BATCH = 16384
BPC = BATCH // NCORES
G3 = BPC // GSZ
RG = 128
NRG = BPC // RG


# --------------------------------------------------------------------------
# host planning
# --------------------------------------------------------------------------

def _balanced_perm(deg, n_pad, n_groups):
    n_real = len(deg)
    order = np.argsort(-deg, kind="stable")
    order = np.concatenate([order, np.arange(n_real, n_pad)])
    gsz = n_pad // n_groups
    pi = np.empty(n_pad, np.int64)
    for r in range(gsz):
        blk = order[r * n_groups:(r + 1) * n_groups]
        cells = np.arange(n_groups) if r % 2 == 0 else \
            np.arange(n_groups - 1, -1, -1)
        pi[blk] = cells * gsz + r
    return pi


def _build_dir_layout(dst_rows, src_rows, vals, groups_per_core, nq, qsize):
    """One slot per edge, sorted (core, group, q, src) for gather locality."""
    g = dst_rows // GSZ
    seg = (dst_rows % GSZ).astype(np.int32)
    q = src_rows // qsize
    srcl = src_rows % qsize
    core = g // groups_per_core
    gl = g % groups_per_core

    sort_key = np.lexsort((srcl, q, gl, core))
    core_s, gl_s = core[sort_key], gl[sort_key]
    q_s, srcl_s = q[sort_key], srcl[sort_key]
    seg_s, val_s = seg[sort_key], vals[sort_key]
    ck = (core_s * groups_per_core + gl_s) * nq + q_s
    nruns = NCORES * groups_per_core * nq
    run_start = np.searchsorted(ck, np.arange(nruns + 1))
    cnt = (run_start[1:] - run_start[:-1]).reshape(
        NCORES, groups_per_core, nq)
    rank = np.arange(len(ck)) - run_start[ck]

    C = np.maximum(1, -(-cnt.max(axis=0) // CH))
    sumC = int(C.sum(axis=1).max())
    for i in range(groups_per_core):
        C[i, nq - 1] += sumC - C[i].sum()
    qoff = np.zeros((groups_per_core, nq + 1), np.int64)
    qoff[:, 1:] = np.cumsum(C, axis=1)

    nslots = sumC * CH
    slot = (qoff[gl_s, q_s] * CH + rank).astype(np.int64)

    srcs = np.zeros((NCORES, groups_per_core, nslots), np.int32)
    srcs[core_s, gl_s, slot] = (q_s * qsize + srcl_s).astype(np.int32)
    pad = np.ones((NCORES, groups_per_core, nslots), bool)
    pad[core_s, gl_s, slot] = False
    c_of = np.arange(nslots) // CH
    qof_slot = np.zeros((groups_per_core, nslots), np.int64)
    for i in range(groups_per_core):
        qq = np.searchsorted(qoff[i], c_of, side="right") - 1
        qof_slot[i] = np.minimum(qq, nq - 1) * qsize
    srcs = np.where(pad, qof_slot[None, :, :], srcs)

    return dict(C=C, sumC=sumC, qoff=qoff, src=srcs,
                e_core=core_s, e_gl=gl_s, e_slot=slot, e_seg=seg_s,
                e_val=val_s, nq=nq, qsize=qsize,
                groups_per_core=groups_per_core)


def _layout_arrays(lay):
    """seg||val stream [NC, gpc, CH, 2*sumC] bf16 and wrapped idx tables."""
    gpc, sumC = lay["groups_per_core"], lay["sumC"]
    nslots = sumC * CH
    segval = np.zeros((NCORES, gpc, CH, 2 * sumC), np.float32)
    p = lay["e_slot"] % CH
    cx = lay["e_slot"] // CH
    segval[lay["e_core"], lay["e_gl"], p, cx] = \
        lay["e_seg"].astype(np.float32)
    segval[lay["e_core"], lay["e_gl"], p, sumC + cx] = lay["e_val"]
    locidx = (lay["src"] % lay["qsize"]).astype(np.int16)
    w = locidx.reshape(NCORES, gpc, nslots // 16, 16)
    w = np.swapaxes(w, 2, 3)
    idx = np.tile(w, (1, 1, 8, 1))
    return segval, idx


def _build_l3_layout(core_e, grp_e, seg_e, srcl_e, val_e, n_groups):
    """Source-sharded pass layout: per-edge owner core, dense group id,
    seg (dst%128), LOCAL src row, value.  nq=1.  Per-group chunk counts C[g]
    are shared across cores (max); groups are ragged (flat column offsets
    qoff).  Returns sv [NC, CH, 2*totC], idx [NC, CH, totC*CH//16]."""
    sort_key = np.lexsort((srcl_e, grp_e, core_e))
    core_s, grp_s = core_e[sort_key], grp_e[sort_key]
    seg_s, srcl_s, val_s = seg_e[sort_key], srcl_e[sort_key], val_e[sort_key]

    ck = core_s * n_groups + grp_s
    run_start = np.searchsorted(ck, np.arange(NCORES * n_groups + 1))
    cnt = (run_start[1:] - run_start[:-1]).reshape(NCORES, n_groups)
    rank = np.arange(len(ck)) - run_start[ck]

    C = np.maximum(1, -(-cnt.max(axis=0) // CH))          # [n_groups]
    qoff = np.zeros(n_groups + 1, np.int64)
    qoff[1:] = np.cumsum(C)
    totC = int(qoff[-1])
    nslots = totC * CH

    slot = (qoff[grp_s] * CH + rank).astype(np.int64)
    srcs = np.zeros((NCORES, nslots), np.int32)
    srcs[core_s, slot] = srcl_s.astype(np.int32)
    # seg/val interleaved per chunk so super-group slices are contiguous
    segv = np.zeros((NCORES, CH, 2 * totC), np.float32)
    p_ = slot % CH
    cx = slot // CH
    segv[core_s, p_, 2 * cx] = seg_s.astype(np.float32)
    segv[core_s, p_, 2 * cx + 1] = val_s

    w = srcs.astype(np.int16).reshape(NCORES, nslots // 16, 16)
    w = np.swapaxes(w, 1, 2)
    idx = np.tile(w, (1, 8, 1))
    return dict(C=C, qoff=qoff, totC=totC, sv=segv, idx=idx,
                n_groups=n_groups)


def _expand_E(lay, table_glob):
    gpc, sumC = lay["groups_per_core"], lay["sumC"]
    E = table_glob[lay["src"]]
    E = E.reshape(NCORES, gpc, sumC, CH, D)
    return np.ascontiguousarray(np.swapaxes(E, 2, 3)).astype(BF)


def _make_plan(user_emb, item_emb, edge_vals, edge_u, edge_i, users, items):
    p = {}
    deg_it = np.bincount(edge_i, minlength=N_IT_REAL)
    deg_us = np.bincount(edge_u, minlength=N_US_REAL)
    pi_it = _balanced_perm(deg_it, N_IT, GI)
    pi_us = _balanced_perm(deg_us, N_US, GU)

    t0_us = np.zeros((N_US, D), np.float32)
    t0_us[pi_us[:N_US_REAL]] = user_emb
    t0_it = np.zeros((N_IT, D), np.float32)
    t0_it[pi_it[:N_IT_REAL]] = item_emb
    p["t0_us"], p["t0_it"] = t0_us, t0_it

    dst_it = pi_it[edge_i]
    dst_us = pi_us[edge_u]
    ev = edge_vals.astype(np.float32)
    p["ui"] = _build_dir_layout(dst_it, dst_us, ev, GI_C, NQ_US, QS)
    p["iu"] = _build_dir_layout(dst_us, dst_it, ev, GU_C, NQ_IT, QS)

    def edges_of(ids_batch, by_node_sorted, node_ptr, other_rows, vals):
        cnts = node_ptr[ids_batch + 1] - node_ptr[ids_batch]
        tot = int(cnts.sum())
        pos_rep = np.repeat(np.arange(len(ids_batch)), cnts)
        starts = np.repeat(node_ptr[ids_batch], cnts)
        within = np.arange(tot) - np.repeat(np.cumsum(cnts) - cnts, cnts)
        eidx = by_node_sorted[starts + within]
        return pos_rep.astype(np.int64), other_rows[eidx], vals[eidx]

    o_i = np.argsort(edge_i, kind="stable")
    ptr_i = np.zeros(N_IT_REAL + 1, np.int64)
    ptr_i[1:] = np.cumsum(deg_it)
    o_u = np.argsort(edge_u, kind="stable")
    ptr_u = np.zeros(N_US_REAL + 1, np.int64)
    ptr_u[1:] = np.cumsum(deg_us)

    # L3 source-sharded: core = owner of the (local) s2 shard row.
    # contrib table rows (DP-padded bf16), core-major blocks of 8192:
    #   [ci3 (s3_i) | cu2 (s2_u at batch users) | cu3 (s3_u) | ci2 (s2_i)]
    SH_US, SH_IT = GU_C * GSZ, GI_C * GSZ

    def contrib_row(part, b):
        return (b // BPC) * (4 * BPC) + part * BPC + (b % BPC)

    # l3a: src table = local s2_u shard; edges: (ci3(b), user row) for all
    # batch items' adjacency + pseudo (cu2(b), row of users[b], val=1).
    posA, srcA, valA = edges_of(items, o_i, ptr_i, dst_us, ev)
    rowsA = contrib_row(0, posA)
    rowsA_p = contrib_row(1, np.arange(BATCH))
    srcA_p = pi_us[users]
    rows_a = np.concatenate([rowsA, rowsA_p])
    srcs_a = np.concatenate([srcA, srcA_p])
    vals_a = np.concatenate([valA, np.ones(BATCH, np.float32)])
    # l3a groups: per core block, first 32 groups (ci3+cu2 sections)
    ga_of_row = (rows_a // (4 * BPC)) * 32 + (rows_a % (4 * BPC)) // GSZ
    p["l3a"] = _build_l3_layout(
        (srcs_a // SH_US).astype(np.int64), ga_of_row,
        (rows_a % GSZ).astype(np.int64), srcs_a % SH_US, vals_a, 32 * NCORES)
    p["glist_a"] = [(blk * (4 * BPC) + j * GSZ)
                    for blk in range(NCORES) for j in range(32)]

    posB, srcB, valB = edges_of(users, o_u, ptr_u, dst_it, ev)
    rowsB = contrib_row(2, posB)
    rowsB_p = contrib_row(3, np.arange(BATCH))
    srcB_p = pi_it[items]
    rows_b = np.concatenate([rowsB, rowsB_p])
    srcs_b = np.concatenate([srcB, srcB_p])
    vals_b = np.concatenate([valB, np.ones(BATCH, np.float32)])
    gb_of_row = (rows_b // (4 * BPC)) * 32 + (rows_b % (4 * BPC) - 2 * BPC) \
        // GSZ
    p["l3b"] = _build_l3_layout(
        (srcs_b // SH_IT).astype(np.int64), gb_of_row,
        (rows_b % GSZ).astype(np.int64), srcs_b % SH_IT, vals_b, 32 * NCORES)
    p["glist_b"] = [(blk * (4 * BPC) + 2 * BPC + j * GSZ)
                    for blk in range(NCORES) for j in range(32)]

    p["bu_rows"] = pi_us[users].reshape(NCORES, BPC)
    p["bi_rows"] = pi_it[items].reshape(NCORES, BPC)
    p["e0u_b"] = user_emb[users].reshape(NCORES, BPC, D).astype(np.float32)
    p["e0i_b"] = item_emb[items].reshape(NCORES, BPC, D).astype(np.float32)
    return p


def _build_device_arrays(p):
    maps = [dict() for _ in range(NCORES)]
    sv_ui, idx_ui = _layout_arrays(p["ui"])
    sv_iu, idx_iu = _layout_arrays(p["iu"])
    sv_3a, idx_3a = p["l3a"]["sv"], p["l3a"]["idx"]
    sv_3b, idx_3b = p["l3b"]["sv"], p["l3b"]["idx"]
    E_ui = _expand_E(p["ui"], p["t0_us"])
    E_iu = _expand_E(p["iu"], p["t0_it"])

    def readout_arrays(rows):
        # one batched gather per table: 2048 quad indices, wrapped 16-wide
        quad = (rows // 4).astype(np.int16)             # [NC, BPC]
        r = (rows.reshape(NCORES, NRG, RG) % 4).astype(np.int64)
        w = quad.reshape(NCORES, BPC // 16, 16)
        w = np.swapaxes(w, 1, 2)                        # [NC, 16, BPC//16]
        idxr = np.tile(w, (1, 8, 1))                    # [NC, 128, BPC//16]
        mask = np.zeros((NCORES, NRG, RG, 4 * DP), BF)
        cc = np.arange(NCORES)[:, None, None]
        gg = np.arange(NRG)[None, :, None]
        kk = np.arange(RG)[None, None, :]
        for d in range(D):
            mask[cc, gg, kk, r * DP + d] = 1.0
        return idxr, mask

    idxr_u, mask_u = readout_arrays(p["bu_rows"])
    idxr_i, mask_i = readout_arrays(p["bi_rows"])

    for c in range(NCORES):
        m = maps[c]
        m["sv_ui"], m["idx_ui"], m["E_ui"] = sv_ui[c], idx_ui[c], E_ui[c]
        m["sv_iu"], m["idx_iu"], m["E_iu"] = sv_iu[c], idx_iu[c], E_iu[c]
        m["sv_3a"], m["idx_3a"] = sv_3a[c], idx_3a[c]
        m["sv_3b"], m["idx_3b"] = sv_3b[c], idx_3b[c]
        m["idxr_u"], m["mask_u"] = idxr_u[c], mask_u[c]
        m["idxr_i"], m["mask_i"] = idxr_i[c], mask_i[c]
        m["e0su"] = p["e0u_b"][c].reshape(NRG, RG, D)
        m["e0si"] = p["e0i_b"][c].reshape(NRG, RG, D)
    return maps


# --------------------------------------------------------------------------
# bass program
# --------------------------------------------------------------------------

def _build_bass(p):
    import concourse.bacc as bacc
    import concourse.tile as tile
    import concourse.mybir as mybir

    from concourse.tile import add_dep_helper

    f32, i16, bf16 = mybir.dt.float32, mybir.dt.int16, mybir.dt.bfloat16
    EQ, MUL = mybir.AluOpType.is_equal, mybir.AluOpType.mult
    nc = bacc.Bacc("TRN2", target_bir_lowering=False, debug=False,
                   num_devices=NCORES)

    def din(name, shape, dt=bf16):
        return nc.dram_tensor(name, list(shape), dt, kind="ExternalInput")

    lays = {}
    for nm, lay in [("ui", p["ui"]), ("iu", p["iu"])]:
        gpc, sumC = lay["groups_per_core"], lay["sumC"]
        t = dict(lay=lay, gpc=gpc, sumC=sumC)
        t["sv"] = din(f"sv_{nm}", [gpc, CH, 2 * sumC], f32)
        t["idx"] = din(f"idx_{nm}", [gpc, CH, sumC * CH // 16], i16)
        t["E"] = din(f"E_{nm}", [gpc, CH, sumC, D])
        lays[nm] = t
    for nm, lay in [("3a", p["l3a"]), ("3b", p["l3b"])]:
        totC = lay["totC"]
        t = dict(C=lay["C"], qoff=lay["qoff"], totC=totC)
        t["sv"] = din(f"sv_{nm}", [CH, 2 * totC], f32)
        t["idx"] = din(f"idx_{nm}", [128, totC * CH // 16], i16)
        lays[nm] = t
    idxr_u = din("idxr_u", [128, BPC // 16], i16)
    idxr_i = din("idxr_i", [128, BPC // 16], i16)
    mask_u = din("mask_u", [NRG, RG, 4 * DP])
    mask_i = din("mask_i", [NRG, RG, 4 * DP])
    e0su = din("e0su", [NRG, RG, D], f32)
    e0si = din("e0si", [NRG, RG, D], f32)
    y_out = nc.dram_tensor("y", [BPC], f32, kind="ExternalOutput")

    reps = [list(range(NCORES))]

    with tile.TileContext(nc) as tc:
        with (
            tc.tile_pool(name="svt", bufs=3) as svp,
            tc.tile_pool(name="oht", bufs=8) as ohp,
            tc.tile_pool(name="gt", bufs=3) as gtp,
            tc.tile_pool(name="ixt", bufs=4) as ixp,
            tc.tile_pool(name="ps", bufs=8, space="PSUM") as psp,
            tc.tile_pool(name="ev", bufs=4) as evp,
            tc.tile_pool(name="ro", bufs=4) as rop,
            tc.tile_pool(name="roq", bufs=1) as roqp,
            tc.tile_pool(name="s3", bufs=1) as s3p,
            tc.tile_pool(name="cst", bufs=1) as cstp,
            tc.tile_pool(name="dram", bufs=1, space="DRAM") as drp,
        ):
            sh = {
                "s1_i": drp.tile([GI_C * GSZ, DP], bf16, name="s1_i_sh"),
                "s1_u": drp.tile([GU_C * GSZ, DP], bf16, name="s1_u_sh"),
                "s2_i": drp.tile([GI_C * GSZ, DP], bf16, name="s2_i_sh"),
                "s2_u": drp.tile([GU_C * GSZ, DP], bf16, name="s2_u_sh"),
            }
            fl = {
                "s1_i": drp.tile([N_IT, DP], bf16, name="s1_i_f"),
                "s1_u": drp.tile([N_US, DP], bf16, name="s1_u_f"),
            }
            contrib = drp.tile([NCORES * 4 * BPC, DP], bf16, name="contrib")
            rs3o = drp.tile([4 * BPC, DP], bf16, name="rs3o")

            iota_t = cstp.tile([128, 128], bf16, name="iota_t")
            nc.gpsimd.iota(iota_t[:], pattern=[[1, 128]], base=0,
                           channel_multiplier=0,
                           allow_small_or_imprecise_dtypes=True)

            ev_tiles = []
            for j in range(4):
                t_ = evp.tile([GSZ, DP], bf16, name=f"evst{j}", tag=f"evst{j}")
                nc.vector.memset(t_[:], 0.0)
                ev_tiles.append(t_)

            anchor = [None]   # last Pool instruction of the previous pass

            def run_pass(t, src_tab, n_src, nq, dst_shard, dst_s3):
                lay, gpc, sumC = t["lay"], t["gpc"], t["sumC"]
                C, qoff = lay["C"], lay["qoff"]
                stream = src_tab is None
                prev_anchor, last_pool = anchor[0], None
                for g in range(gpc):
                    sv = svp.tile([CH, 2 * sumC], mybir.dt.float32,
                                  name="sv", tag="sv")
                    nc.sync.dma_start(sv[:], t["sv"].ap()[g])
                    if stream:
                        gt = gtp.tile([CH, sumC, D], bf16, name="gts",
                                      tag="gts")
                        nc.sync.dma_start(gt[:], t["E"].ap()[g])
                        rhs = lambda c: gt[:, c, :]
                    else:
                        gt = gtp.tile([CH, sumC, DP], bf16, name="gtg",
                                      tag="gtg")
                        ixt = ixp.tile([CH, sumC * CH // 16], i16,
                                       name="ixt", tag="ixt")
                        nc.sync.dma_start(ixt[:], t["idx"].ap()[g])
                        for q in range(nq):
                            cq, off = int(C[g, q]), int(qoff[g, q])
                            if cq == 0:
                                continue
                            qlo = q * QS
                            qhi = min((q + 1) * QS, n_src)
                            gi = nc.gpsimd.dma_gather(
                                gt[:, off:off + cq, :],
                                src_tab.opt()[qlo:qhi],
                                ixt[:, off * 8:(off + cq) * 8],
                                cq * CH, cq * CH, DP,
                                single_packet=False,
                            )
                            # Pin early gathers behind the previous pass's
                            # Pool tail: a hoisted gather parks on Pool.SEQ
                            # waiting for the AllGather and head-of-line
                            # blocks every later Pool instruction.
                            if g < 4 and prev_anchor is not None:
                                add_dep_helper(gi.ins, prev_anchor.ins,
                                               reason="pool queue order")
                            last_pool = gi
                        rhs = lambda c: gt[:, c, 0:D]
                    ps = psp.tile([GSZ, D], mybir.dt.float32, name="ps",
                                  tag="ps", space="PSUM")
                    for cx in range(sumC):
                        oh = ohp.tile([CH, GSZ], bf16, name="oh", tag="oh")
                        # Pool takes a third of the stream-pass one-hots
                        # (it is ~3x slower per op and busy with gathers in
                        # gather passes).
                        eng = nc.gpsimd if (stream and cx % 3 == 2) \
                            else nc.vector
                        ts_i = eng.tensor_scalar(
                            out=oh[:], in0=iota_t[:],
                            scalar1=sv[:, cx:cx + 1],
                            scalar2=sv[:, sumC + cx:sumC + cx + 1],
                            op0=EQ, op1=MUL)
                        if eng is nc.gpsimd:
                            last_pool = ts_i
                        nc.tensor.matmul(ps[:], lhsT=oh[:],
                                         rhs=rhs(cx), start=(cx == 0),
                                         stop=(cx == sumC - 1))
                    if dst_s3 is None:
                        ev = ev_tiles[g % 4]
                        nc.scalar.copy(ev[:, 0:D], ps[:])
                        nc.sync.dma_start(
                            dst_shard.opt()[g * GSZ:(g + 1) * GSZ, :], ev[:])
                    else:
                        nc.scalar.copy(dst_s3[:, g, :], ps[:])
                if last_pool is not None:
                    anchor[0] = last_pool

            def run_l3(t, sv_in, idx_in, src_tab, qsize, glist):
                """Source-sharded L3 pass: ragged per-group chunk counts,
                super-gathers over SG consecutive groups from the LOCAL s2
                shard, contrib-table destination rows."""
                C, qoff, totC = t["C"], t["qoff"], t["totC"]
                nG = len(glist)
                SG = 8
                prev_anchor, last_pool = anchor[0], None
                for si in range(0, nG, SG):
                    glo, ghi = si, min(nG, si + SG)
                    clo, chi = int(qoff[glo]), int(qoff[ghi])
                    cq = chi - clo
                    svt = svp.tile([CH, 2 * cq], mybir.dt.float32,
                                   name="sv3", tag="sv")
                    nc.sync.dma_start(svt[:],
                                      sv_in.ap()[:, 2 * clo:2 * chi])
                    ixt = ixp.tile([128, cq * 8], i16, name="ixt3",
                                   tag="ixt")
                    nc.sync.dma_start(ixt[:],
                                      idx_in.ap()[:, clo * 8:chi * 8])
                    gt = gtp.tile([CH, cq, DP], bf16, name="gt3", tag="gtg")
                    gi = nc.gpsimd.dma_gather(
                        gt[:], src_tab.opt()[0:qsize], ixt[:],
                        cq * CH, cq * CH, DP, single_packet=False)
                    if si < 4 * SG and prev_anchor is not None:
                        add_dep_helper(gi.ins, prev_anchor.ins,
                                       reason="pool queue order")
                    last_pool = gi
                    for g in range(glo, ghi):
                        ps = psp.tile([GSZ, D], mybir.dt.float32, name="ps",
                                      tag="ps", space="PSUM")
                        cg = int(C[g])
                        for k in range(cg):
                            cx = int(qoff[g]) + k - clo
                            oh = ohp.tile([CH, GSZ], bf16, name="oh",
                                          tag="oh")
                            nc.vector.tensor_scalar(
                                out=oh[:], in0=iota_t[:],
                                scalar1=svt[:, 2 * cx:2 * cx + 1],
                                scalar2=svt[:, 2 * cx + 1:2 * cx + 2],
                                op0=EQ, op1=MUL)
                            nc.tensor.matmul(ps[:], lhsT=oh[:],
                                             rhs=gt[:, cx, 0:D],
                                             start=(k == 0),
                                             stop=(k == cg - 1))
                        ev = ev_tiles[g % 4]
                        nc.scalar.copy(ev[:, 0:D], ps[:])
                        nc.sync.dma_start(
                            contrib.opt()[glist[g]:glist[g] + GSZ, :], ev[:])
                if last_pool is not None:
                    anchor[0] = last_pool

            def ag(shard, full):
                nc.gpsimd.collective_compute(
                    "AllGather", mybir.AluOpType.bypass, replica_groups=reps,
                    ins=[shard.opt()], outs=[full.opt()])

            # AG(s1_i) overlaps iu-L1; AG(s1_u) overlaps iu-L2 (which
            # consumes s1_i).  L2 writes stay shard-local; L3 is
            # source-sharded and gathers from the LOCAL s2 shards, writing
            # batch contributions that a single ReduceScatter sums.
            run_pass(lays["ui"], None, 0, 0, sh["s1_i"], None)
            ag(sh["s1_i"], fl["s1_i"])
            run_pass(lays["iu"], None, 0, 0, sh["s1_u"], None)
            ag(sh["s1_u"], fl["s1_u"])
            run_pass(lays["iu"], fl["s1_i"], N_IT, NQ_IT, sh["s2_u"], None)
            run_pass(lays["ui"], fl["s1_u"], N_US, NQ_US, sh["s2_i"], None)
            run_l3(lays["3a"], lays["3a"]["sv"], lays["3a"]["idx"],
                   sh["s2_u"], GU_C * GSZ, p["glist_a"])
            run_l3(lays["3b"], lays["3b"]["sv"], lays["3b"]["idx"],
                   sh["s2_i"], GI_C * GSZ, p["glist_b"])
            nc.gpsimd.collective_compute(
                "ReduceScatter", mybir.AluOpType.add, replica_groups=reps,
                ins=[contrib.opt()], outs=[rs3o.opt()])

            qv = {k: fl[k].opt().rearrange("(n r) d -> n (r d)", r=4)
                  for k in fl}

            # batched readout quad gathers for the layer-1 terms
            gq = {}
            for nm, idxr in (("s1_u", idxr_u), ("s1_i", idxr_i)):
                ixr = roqp.tile([128, BPC // 16], i16, name=f"ixr_{nm}")
                nc.sync.dma_start(ixr[:], idxr.ap())
                gq[nm] = roqp.tile([RG, NRG, 4 * DP], bf16, name=f"gq_{nm}")
                gi = nc.gpsimd.dma_gather(gq[nm][:], qv[nm], ixr[:], BPC,
                                          BPC, 4 * DP, single_packet=False)
                if anchor[0] is not None:
                    add_dep_helper(gi.ins, anchor[0].ins,
                                   reason="pool queue order")

            def side(rg, maskt, g1, e0t, part_s2, part_s3):
                mk = rop.tile([RG, 4 * DP], bf16, name="mk", tag="mk")
                nc.sync.dma_start(mk[:], maskt.ap()[rg])
                e0 = rop.tile([RG, D], mybir.dt.float32, name="e0", tag="e0")
                nc.sync.dma_start(e0[:], e0t.ap()[rg])
                acc = rop.tile([RG, D], mybir.dt.float32, name="acc",
                               tag="acc")
                r2 = rop.tile([RG, DP], bf16, name="r2", tag="r2")
                nc.sync.dma_start(
                    r2[:], rs3o.opt()[part_s2 * BPC + rg * RG:
                                      part_s2 * BPC + (rg + 1) * RG, :])
                r3 = rop.tile([RG, DP], bf16, name="r3", tag="r3")
                nc.sync.dma_start(
                    r3[:], rs3o.opt()[part_s3 * BPC + rg * RG:
                                      part_s3 * BPC + (rg + 1) * RG, :])
                nc.vector.tensor_add(out=acc[:], in0=e0[:], in1=r2[:, 0:D])
                nc.vector.tensor_add(out=acc[:], in0=acc[:], in1=r3[:, 0:D])
                sel = rop.tile([RG, 4 * DP], mybir.dt.float32,
                               name="sel", tag="sel")
                nc.vector.tensor_mul(out=sel[:], in0=g1[:, rg, :],
                                     in1=mk[:])
                red = rop.tile([RG, D], mybir.dt.float32, name="red",
                               tag="red")
                nc.vector.reduce_sum(
                    red[:],
                    sel[:].rearrange("p (r d) -> p d r", r=4)[:, 0:D, :],
                    axis=mybir.AxisListType.X)
                nc.vector.tensor_add(out=acc[:], in0=acc[:], in1=red[:])
                return acc

            yv = y_out.ap().rearrange("(g p) -> g p", p=RG)
            for rg in range(NRG):
                # contrib parts per core block: [ci3 | cu2 | cu3 | ci2]
                su = side(rg, mask_u, gq["s1_u"], e0su, 1, 2)
                si = side(rg, mask_i, gq["s1_i"], e0si, 3, 0)
                pr = rop.tile([RG, D], mybir.dt.float32, name="pr", tag="pr")
                nc.vector.tensor_mul(out=pr[:], in0=su[:], in1=si[:])
                nc.vector.tensor_scalar_mul(out=pr[:], in0=pr[:],
                                            scalar1=1.0 / 16.0)
                yc = rop.tile([RG, 1], mybir.dt.float32, name="yc", tag="yc")
                nc.vector.reduce_sum(yc[:], pr[:], axis=mybir.AxisListType.X)
                nc.sync.dma_start(yv[rg], yc[:, 0])

    nc.compile()
    return nc


_CACHE = {}
_TRACE = False
_TRACE_DIR = None
_LAST_RES = None


def _schedule_key(p):
    import hashlib
    h = hashlib.sha1()
    for k in ("ui", "iu"):
        h.update(p[k]["C"].tobytes())
        h.update(np.int64(p[k]["sumC"]).tobytes())
    for k in ("l3a", "l3b"):
        h.update(p[k]["C"].tobytes())
        h.update(np.int64(p[k]["totC"]).tobytes())
    return h.hexdigest()


def kernel(user_emb, item_emb, edge_vals, edge_u, edge_i, users, items):
    global _LAST_RES
    from concourse.bass_utils import run_bass_kernel_spmd

    user_emb = np.asarray(user_emb, np.float32)
    item_emb = np.asarray(item_emb, np.float32)
    edge_vals = np.asarray(edge_vals, np.float32)
    edge_u = np.asarray(edge_u, np.int64)
    edge_i = np.asarray(edge_i, np.int64)
    users = np.asarray(users, np.int64)
    items = np.asarray(items, np.int64)

    p = _make_plan(user_emb, item_emb, edge_vals, edge_u, edge_i, users,
                   items)
    maps = _build_device_arrays(p)
    key = _schedule_key(p)
    if _CACHE.get("key") != key:
        _CACHE["nc"] = _build_bass(p)
        _CACHE["key"] = key
    nc = _CACHE["nc"]
    res = run_bass_kernel_spmd(nc, maps, core_ids=list(range(NCORES)),
                               trace=_TRACE, tmpdir=_TRACE_DIR)
    _LAST_RES = res
    y = np.concatenate([res.results[c]["y"] for c in range(NCORES)])
    return y.astype(np.float32)



# revision 16
# speedup vs baseline: 1.0991x; 1.0991x over previous
"""CredLightGCN (3-layer LightGCN propagation + batch dot readout) on 8
Trainium2 NeuronCores.

Strategy (all sizes hardcoded for the nn_CredLightGCN problem):
  * The six SpMMs (2 directions x 3 layers) are computed as PE one-hot
    matmuls: for each destination group of 128 rows, PSUM accumulates
    chunks  out[seg,d] += OH[slot,seg]^T @ G[slot,d]  where OH is a
    one-hot (edge-value-weighted) selection matrix generated ON DEVICE by
    the vector engines:  OH[p, f] = (iota[f] == seg[p]) * val[p]  via a
    fused tensor_scalar(is_equal, mult) with per-partition scalars.  Only
    the tiny seg/val streams (4B/edge) come from HBM, not the 32KB/chunk
    dense M tiles.  One-hot generation alternates DVE / Pool to balance
    engine load.  One slot per edge (no dedup).
  * Layer 1 needs no on-device gathers: G streams from host-expanded edge
    tables (the inputs are known on the host).
  * Layer 2 gathers source rows with gpsimd dma_gather (256B rows, int16
    indices, tables split in 25088-row windows).
  * Layer 3 is batch-funneled: only rows reachable from the 16384 query
    pairs are produced (batch positions are the destination rows).
  * Tables are bf16 padded to 128 cols (256B rows) to satisfy dma_gather's
    256B element constraint; PSUM accumulation stays f32.
  * Cores own disjoint destination-row shards; full tables are rebuilt
    between passes with DRAM AllGather collectives.
    Pass order (ui-L1, iu-L1, iu-L2, ui-L2, l3a, l3b) is chosen
    so every AllGather overlaps the pass that follows it; the next
    gather-pass's first gathers carry explicit deps on the previous
    pass's Pool tail so a gather parked on its AllGather wait cannot
    head-of-line block Pool.SEQ.
  * Readout: gather s1/s2 rows as 1KB "quad" rows (4 padded rows per
    descriptor, index = row//4) in ONE dma_gather per (side, table),
    select the right sub-row with a bf16 mask + axis reduce, add the
    layer-0 and layer-3 terms, multiply sides and row-reduce.

Row permutation: items/users are assigned to device rows by a
degree-balanced snake so every destination group has a near-equal edge
count, which makes the chunk schedule uniform across the 8 cores (all
cores run one shared program; per-core data differs).
"""

import numpy as np
import ml_dtypes

NCORES = 8
GSZ = 128         # dst rows per group (PSUM partitions)
CH = 128          # edge slots per chunk (PE contraction K)
D = 64            # embedding dim
DP = 128          # padded bf16 row width (256B)
BF = ml_dtypes.bfloat16

N_IT_REAL, N_US_REAL = 50000, 100000
UNIT = NCORES * GSZ
N_IT = -(-N_IT_REAL // UNIT) * UNIT          # 50176
N_US = -(-N_US_REAL // UNIT) * UNIT          # 100352
GI, GU = N_IT // GSZ, N_US // GSZ
GI_C, GU_C = GI // NCORES, GU // NCORES
QS = 25088
NQ_US, NQ_IT = -(-N_US // QS), -(-N_IT // QS)    # 4, 2
BATCH = 16384
BPC = BATCH // NCORES
G3 = BPC // GSZ
RG = 128
NRG = BPC // RG


# --------------------------------------------------------------------------
# host planning
# --------------------------------------------------------------------------

def _balanced_perm(deg, n_pad, n_groups):
    n_real = len(deg)
    order = np.argsort(-deg, kind="stable")
    order = np.concatenate([order, np.arange(n_real, n_pad)])
    gsz = n_pad // n_groups
    pi = np.empty(n_pad, np.int64)
    for r in range(gsz):
        blk = order[r * n_groups:(r + 1) * n_groups]
        cells = np.arange(n_groups) if r % 2 == 0 else \
            np.arange(n_groups - 1, -1, -1)
        pi[blk] = cells * gsz + r
    return pi


def _build_dir_layout(dst_rows, src_rows, vals, groups_per_core, nq, qsize):
    """One slot per edge, sorted (core, group, q, src) for gather locality."""
    g = dst_rows // GSZ
    seg = (dst_rows % GSZ).astype(np.int32)
    q = src_rows // qsize
    srcl = src_rows % qsize
    core = g // groups_per_core
    gl = g % groups_per_core

    sort_key = np.lexsort((srcl, q, gl, core))
    core_s, gl_s = core[sort_key], gl[sort_key]
    q_s, srcl_s = q[sort_key], srcl[sort_key]
    seg_s, val_s = seg[sort_key], vals[sort_key]
    ck = (core_s * groups_per_core + gl_s) * nq + q_s
    nruns = NCORES * groups_per_core * nq
    run_start = np.searchsorted(ck, np.arange(nruns + 1))
    cnt = (run_start[1:] - run_start[:-1]).reshape(
        NCORES, groups_per_core, nq)
    rank = np.arange(len(ck)) - run_start[ck]

    C = np.maximum(1, -(-cnt.max(axis=0) // CH))
    sumC = int(C.sum(axis=1).max())
    for i in range(groups_per_core):
        C[i, nq - 1] += sumC - C[i].sum()
    qoff = np.zeros((groups_per_core, nq + 1), np.int64)
    qoff[:, 1:] = np.cumsum(C, axis=1)

    nslots = sumC * CH
    slot = (qoff[gl_s, q_s] * CH + rank).astype(np.int64)

    srcs = np.zeros((NCORES, groups_per_core, nslots), np.int32)
    srcs[core_s, gl_s, slot] = (q_s * qsize + srcl_s).astype(np.int32)
    pad = np.ones((NCORES, groups_per_core, nslots), bool)
    pad[core_s, gl_s, slot] = False
    c_of = np.arange(nslots) // CH
    qof_slot = np.zeros((groups_per_core, nslots), np.int64)
    for i in range(groups_per_core):
        qq = np.searchsorted(qoff[i], c_of, side="right") - 1
        qof_slot[i] = np.minimum(qq, nq - 1) * qsize
    srcs = np.where(pad, qof_slot[None, :, :], srcs)

    return dict(C=C, sumC=sumC, qoff=qoff, src=srcs,
                e_core=core_s, e_gl=gl_s, e_slot=slot, e_seg=seg_s,
                e_val=val_s, nq=nq, qsize=qsize,
                groups_per_core=groups_per_core)


def _layout_arrays(lay):
    """seg||val stream [NC, gpc, CH, 2*sumC] bf16 and wrapped idx tables."""
    gpc, sumC = lay["groups_per_core"], lay["sumC"]
    nslots = sumC * CH
    segval = np.zeros((NCORES, gpc, CH, 2 * sumC), np.float32)
    p = lay["e_slot"] % CH
    cx = lay["e_slot"] // CH
    segval[lay["e_core"], lay["e_gl"], p, cx] = \
        lay["e_seg"].astype(np.float32)
    segval[lay["e_core"], lay["e_gl"], p, sumC + cx] = lay["e_val"]
    locidx = (lay["src"] % lay["qsize"]).astype(np.int16)
    w = locidx.reshape(NCORES, gpc, nslots // 16, 16)
    w = np.swapaxes(w, 2, 3)
    idx = np.tile(w, (1, 1, 8, 1))
    return segval, idx


def _build_l3_layout(core_e, grp_e, seg_e, srcl_e, val_e, n_groups):
    """Source-sharded pass layout: per-edge owner core, dense group id,
    seg (dst%128), LOCAL src row, value.  nq=1.  Per-group chunk counts C[g]
    are shared across cores (max); groups are ragged (flat column offsets
    qoff).  Returns sv [NC, CH, 2*totC], idx [NC, CH, totC*CH//16]."""
    sort_key = np.lexsort((srcl_e, grp_e, core_e))
    core_s, grp_s = core_e[sort_key], grp_e[sort_key]
    seg_s, srcl_s, val_s = seg_e[sort_key], srcl_e[sort_key], val_e[sort_key]

    ck = core_s * n_groups + grp_s
    run_start = np.searchsorted(ck, np.arange(NCORES * n_groups + 1))
    cnt = (run_start[1:] - run_start[:-1]).reshape(NCORES, n_groups)
    rank = np.arange(len(ck)) - run_start[ck]

    C = np.maximum(1, -(-cnt.max(axis=0) // CH))          # [n_groups]
    qoff = np.zeros(n_groups + 1, np.int64)
    qoff[1:] = np.cumsum(C)
    totC = int(qoff[-1])
    nslots = totC * CH

    slot = (qoff[grp_s] * CH + rank).astype(np.int64)
    srcs = np.zeros((NCORES, nslots), np.int32)
    srcs[core_s, slot] = srcl_s.astype(np.int32)
    # seg/val interleaved per chunk so super-group slices are contiguous
    segv = np.zeros((NCORES, CH, 2 * totC), np.float32)
    p_ = slot % CH
    cx = slot // CH
    segv[core_s, p_, 2 * cx] = seg_s.astype(np.float32)
    segv[core_s, p_, 2 * cx + 1] = val_s

    w = srcs.astype(np.int16).reshape(NCORES, nslots // 16, 16)
    w = np.swapaxes(w, 1, 2)
    idx = np.tile(w, (1, 8, 1))
    return dict(C=C, qoff=qoff, totC=totC, sv=segv, idx=idx,
                n_groups=n_groups)


def _expand_E(lay, table_glob):
    gpc, sumC = lay["groups_per_core"], lay["sumC"]
    E = table_glob[lay["src"]]
    E = E.reshape(NCORES, gpc, sumC, CH, D)
    return np.ascontiguousarray(np.swapaxes(E, 2, 3)).astype(BF)


def _make_plan(user_emb, item_emb, edge_vals, edge_u, edge_i, users, items):
    p = {}
    deg_it = np.bincount(edge_i, minlength=N_IT_REAL)
    deg_us = np.bincount(edge_u, minlength=N_US_REAL)
    pi_it = _balanced_perm(deg_it, N_IT, GI)
    pi_us = _balanced_perm(deg_us, N_US, GU)

    t0_us = np.zeros((N_US, D), np.float32)
    t0_us[pi_us[:N_US_REAL]] = user_emb
    t0_it = np.zeros((N_IT, D), np.float32)
    t0_it[pi_it[:N_IT_REAL]] = item_emb
    p["t0_us"], p["t0_it"] = t0_us, t0_it

    dst_it = pi_it[edge_i]
    dst_us = pi_us[edge_u]
    ev = edge_vals.astype(np.float32)
    p["ui"] = _build_dir_layout(dst_it, dst_us, ev, GI_C, NQ_US, QS)
    p["iu"] = _build_dir_layout(dst_us, dst_it, ev, GU_C, NQ_IT, QS)

    def edges_of(ids_batch, by_node_sorted, node_ptr, other_rows, vals):
        cnts = node_ptr[ids_batch + 1] - node_ptr[ids_batch]
        tot = int(cnts.sum())
        pos_rep = np.repeat(np.arange(len(ids_batch)), cnts)
        starts = np.repeat(node_ptr[ids_batch], cnts)
        within = np.arange(tot) - np.repeat(np.cumsum(cnts) - cnts, cnts)
        eidx = by_node_sorted[starts + within]
        return pos_rep.astype(np.int64), other_rows[eidx], vals[eidx]

    o_i = np.argsort(edge_i, kind="stable")
    ptr_i = np.zeros(N_IT_REAL + 1, np.int64)
    ptr_i[1:] = np.cumsum(deg_it)
    o_u = np.argsort(edge_u, kind="stable")
    ptr_u = np.zeros(N_US_REAL + 1, np.int64)
    ptr_u[1:] = np.cumsum(deg_us)

    # L3 source-sharded: core = owner of the (local) s2 shard row.
    # contrib table rows (DP-padded bf16), core-major blocks of 8192:
    #   [ci3 (s3_i) | cu2 (s2_u at batch users) | cu3 (s3_u) | ci2 (s2_i)]
    SH_US, SH_IT = GU_C * GSZ, GI_C * GSZ

    def contrib_row(part, b):
        return (b // BPC) * (4 * BPC) + part * BPC + (b % BPC)

    # l3a: src table = local s2_u shard; edges: (ci3(b), user row) for all
    # batch items' adjacency + pseudo (cu2(b), row of users[b], val=1).
    posA, srcA, valA = edges_of(items, o_i, ptr_i, dst_us, ev)
    rowsA = contrib_row(0, posA)
    rowsA_p = contrib_row(1, np.arange(BATCH))
    srcA_p = pi_us[users]
    rows_a = np.concatenate([rowsA, rowsA_p])
    srcs_a = np.concatenate([srcA, srcA_p])
    vals_a = np.concatenate([valA, np.ones(BATCH, np.float32)])
    # l3a groups: per core block, first 32 groups (ci3+cu2 sections)
    ga_of_row = (rows_a // (4 * BPC)) * 32 + (rows_a % (4 * BPC)) // GSZ
    p["l3a"] = _build_l3_layout(
        (srcs_a // SH_US).astype(np.int64), ga_of_row,
        (rows_a % GSZ).astype(np.int64), srcs_a % SH_US, vals_a, 32 * NCORES)
    p["glist_a"] = [(blk * (4 * BPC) + j * GSZ)
                    for blk in range(NCORES) for j in range(32)]

    posB, srcB, valB = edges_of(users, o_u, ptr_u, dst_it, ev)
    rowsB = contrib_row(2, posB)
    rowsB_p = contrib_row(3, np.arange(BATCH))
    srcB_p = pi_it[items]
    rows_b = np.concatenate([rowsB, rowsB_p])
    srcs_b = np.concatenate([srcB, srcB_p])
    vals_b = np.concatenate([valB, np.ones(BATCH, np.float32)])
    gb_of_row = (rows_b // (4 * BPC)) * 32 + (rows_b % (4 * BPC) - 2 * BPC) \
        // GSZ
    p["l3b"] = _build_l3_layout(
        (srcs_b // SH_IT).astype(np.int64), gb_of_row,
        (rows_b % GSZ).astype(np.int64), srcs_b % SH_IT, vals_b, 32 * NCORES)
    p["glist_b"] = [(blk * (4 * BPC) + 2 * BPC + j * GSZ)
                    for blk in range(NCORES) for j in range(32)]

    p["bu_rows"] = pi_us[users].reshape(NCORES, BPC)
    p["bi_rows"] = pi_it[items].reshape(NCORES, BPC)
    p["e0u_b"] = user_emb[users].reshape(NCORES, BPC, D).astype(np.float32)
    p["e0i_b"] = item_emb[items].reshape(NCORES, BPC, D).astype(np.float32)
    return p


def _build_device_arrays(p):
    maps = [dict() for _ in range(NCORES)]
    sv_ui, idx_ui = _layout_arrays(p["ui"])
    sv_iu, idx_iu = _layout_arrays(p["iu"])
    sv_3a, idx_3a = p["l3a"]["sv"], p["l3a"]["idx"]
    sv_3b, idx_3b = p["l3b"]["sv"], p["l3b"]["idx"]
    E_ui = _expand_E(p["ui"], p["t0_us"])
    E_iu = _expand_E(p["iu"], p["t0_it"])

    def readout_arrays(rows):
        # one batched gather per table: 2048 quad indices, wrapped 16-wide
        quad = (rows // 4).astype(np.int16)             # [NC, BPC]
        r = (rows.reshape(NCORES, NRG, RG) % 4).astype(np.int64)
        w = quad.reshape(NCORES, BPC // 16, 16)
        w = np.swapaxes(w, 1, 2)                        # [NC, 16, BPC//16]
        idxr = np.tile(w, (1, 8, 1))                    # [NC, 128, BPC//16]
        mask = np.zeros((NCORES, NRG, RG, 4 * DP), BF)
        cc = np.arange(NCORES)[:, None, None]
        gg = np.arange(NRG)[None, :, None]
        kk = np.arange(RG)[None, None, :]
        for d in range(D):
            mask[cc, gg, kk, r * DP + d] = 1.0
        return idxr, mask

    idxr_u, mask_u = readout_arrays(p["bu_rows"])
    idxr_i, mask_i = readout_arrays(p["bi_rows"])

    for c in range(NCORES):
        m = maps[c]
        m["sv_ui"], m["idx_ui"], m["E_ui"] = sv_ui[c], idx_ui[c], E_ui[c]
        m["sv_iu"], m["idx_iu"], m["E_iu"] = sv_iu[c], idx_iu[c], E_iu[c]
        m["sv_3a"], m["idx_3a"] = sv_3a[c], idx_3a[c]
        m["sv_3b"], m["idx_3b"] = sv_3b[c], idx_3b[c]
        m["idxr_u"], m["mask_u"] = idxr_u[c], mask_u[c]
        m["idxr_i"], m["mask_i"] = idxr_i[c], mask_i[c]
        m["e0su"] = p["e0u_b"][c].reshape(NRG, RG, D)
        m["e0si"] = p["e0i_b"][c].reshape(NRG, RG, D)
    return maps


# --------------------------------------------------------------------------
# bass program
# --------------------------------------------------------------------------

def _build_bass(p):
    import concourse.bacc as bacc
    import concourse.tile as tile
    import concourse.mybir as mybir

    from concourse.tile import add_dep_helper

    f32, i16, bf16 = mybir.dt.float32, mybir.dt.int16, mybir.dt.bfloat16
    EQ, MUL = mybir.AluOpType.is_equal, mybir.AluOpType.mult
    nc = bacc.Bacc("TRN2", target_bir_lowering=False, debug=False,
                   num_devices=NCORES)

    def din(name, shape, dt=bf16):
        return nc.dram_tensor(name, list(shape), dt, kind="ExternalInput")

    lays = {}
    for nm, lay in [("ui", p["ui"]), ("iu", p["iu"])]:
        gpc, sumC = lay["groups_per_core"], lay["sumC"]
        t = dict(lay=lay, gpc=gpc, sumC=sumC)
        t["sv"] = din(f"sv_{nm}", [gpc, CH, 2 * sumC], f32)
        t["idx"] = din(f"idx_{nm}", [gpc, CH, sumC * CH // 16], i16)
        t["E"] = din(f"E_{nm}", [gpc, CH, sumC, D])
        lays[nm] = t
    for nm, lay in [("3a", p["l3a"]), ("3b", p["l3b"])]:
        totC = lay["totC"]
        t = dict(C=lay["C"], qoff=lay["qoff"], totC=totC)
        t["sv"] = din(f"sv_{nm}", [CH, 2 * totC], f32)
        t["idx"] = din(f"idx_{nm}", [128, totC * CH // 16], i16)
        lays[nm] = t
    idxr_u = din("idxr_u", [128, BPC // 16], i16)
    idxr_i = din("idxr_i", [128, BPC // 16], i16)
    mask_u = din("mask_u", [NRG, RG, 4 * DP])
    mask_i = din("mask_i", [NRG, RG, 4 * DP])
    e0su = din("e0su", [NRG, RG, D], f32)
    e0si = din("e0si", [NRG, RG, D], f32)
    y_out = nc.dram_tensor("y", [BPC], f32, kind="ExternalOutput")

    reps = [list(range(NCORES))]

    with tile.TileContext(nc) as tc:
        with (
            tc.tile_pool(name="svt", bufs=3) as svp,
            tc.tile_pool(name="oht", bufs=8) as ohp,
            tc.tile_pool(name="gt", bufs=3) as gtp,
            tc.tile_pool(name="ixt", bufs=4) as ixp,
            tc.tile_pool(name="ps", bufs=8, space="PSUM") as psp,
            tc.tile_pool(name="ev", bufs=4) as evp,
            tc.tile_pool(name="ro", bufs=4) as rop,
            tc.tile_pool(name="roq", bufs=1) as roqp,
            tc.tile_pool(name="s3", bufs=1) as s3p,
            tc.tile_pool(name="cst", bufs=1) as cstp,
            tc.tile_pool(name="dram", bufs=1, space="DRAM") as drp,
        ):
            sh = {
                "s1_i": drp.tile([GI_C * GSZ, DP], bf16, name="s1_i_sh"),
                "s1_u": drp.tile([GU_C * GSZ, DP], bf16, name="s1_u_sh"),
                "s2_i": drp.tile([GI_C * GSZ, DP], bf16, name="s2_i_sh"),
                "s2_u": drp.tile([GU_C * GSZ, DP], bf16, name="s2_u_sh"),
            }
            fl = {
                "s1_i": drp.tile([N_IT, DP], bf16, name="s1_i_f"),
                "s1_u": drp.tile([N_US, DP], bf16, name="s1_u_f"),
            }
            contrib = drp.tile([NCORES * 4 * BPC, DP], bf16, name="contrib")
            rs3o = drp.tile([4 * BPC, DP], bf16, name="rs3o")

            iota_t = cstp.tile([128, 128], bf16, name="iota_t")
            nc.gpsimd.iota(iota_t[:], pattern=[[1, 128]], base=0,
                           channel_multiplier=0,
                           allow_small_or_imprecise_dtypes=True)

            ev_tiles = []
            for j in range(4):
                t_ = evp.tile([GSZ, DP], bf16, name=f"evst{j}", tag=f"evst{j}")
                nc.vector.memset(t_[:], 0.0)
                ev_tiles.append(t_)

            anchor = [None]   # last Pool instruction of the previous pass

            def run_pass(t, src_tab, n_src, nq, dst_shard, dst_s3,
                         hook=None, hook_skew=8, hook_every=1):
                lay, gpc, sumC = t["lay"], t["gpc"], t["sumC"]
                C, qoff = lay["C"], lay["qoff"]
                stream = src_tab is None
                prev_anchor, last_pool = anchor[0], None
                for g in range(gpc):
                    if hook is not None and g >= hook_skew and \
                            (g - hook_skew) % hook_every == 0:
                        next(hook, None)
                    sv = svp.tile([CH, 2 * sumC], mybir.dt.float32,
                                  name="sv", tag="sv")
                    nc.sync.dma_start(sv[:], t["sv"].ap()[g])
                    if stream:
                        gt = gtp.tile([CH, sumC, D], bf16, name="gts",
                                      tag="gts")
                        nc.sync.dma_start(gt[:], t["E"].ap()[g])
                        rhs = lambda c: gt[:, c, :]
                    else:
                        gt = gtp.tile([CH, sumC, DP], bf16, name="gtg",
                                      tag="gtg")
                        ixt = ixp.tile([CH, sumC * CH // 16], i16,
                                       name="ixt", tag="ixt")
                        nc.sync.dma_start(ixt[:], t["idx"].ap()[g])
                        for q in range(nq):
                            cq, off = int(C[g, q]), int(qoff[g, q])
                            if cq == 0:
                                continue
                            qlo = q * QS
                            qhi = min((q + 1) * QS, n_src)
                            gi = nc.gpsimd.dma_gather(
                                gt[:, off:off + cq, :],
                                src_tab.opt()[qlo:qhi],
                                ixt[:, off * 8:(off + cq) * 8],
                                cq * CH, cq * CH, DP,
                                single_packet=False,
                            )
                            # Pin early gathers behind the previous pass's
                            # Pool tail: a hoisted gather parks on Pool.SEQ
                            # waiting for the AllGather and head-of-line
                            # blocks every later Pool instruction.
                            if g < 4 and prev_anchor is not None:
                                add_dep_helper(gi.ins, prev_anchor.ins,
                                               reason="pool queue order")
                            last_pool = gi
                        rhs = lambda c: gt[:, c, 0:D]
                    ps = psp.tile([GSZ, D], mybir.dt.float32, name="ps",
                                  tag="ps", space="PSUM")
                    for cx in range(sumC):
                        oh = ohp.tile([CH, GSZ], bf16, name="oh", tag="oh")
                        # Pool takes a third of the stream-pass one-hots
                        # (it is ~3x slower per op and busy with gathers in
                        # gather passes).
                        eng = nc.gpsimd if (stream and cx % 3 == 2) \
                            else nc.vector
                        ts_i = eng.tensor_scalar(
                            out=oh[:], in0=iota_t[:],
                            scalar1=sv[:, cx:cx + 1],
                            scalar2=sv[:, sumC + cx:sumC + cx + 1],
                            op0=EQ, op1=MUL)
                        if eng is nc.gpsimd:
                            last_pool = ts_i
                        nc.tensor.matmul(ps[:], lhsT=oh[:],
                                         rhs=rhs(cx), start=(cx == 0),
                                         stop=(cx == sumC - 1))
                    if dst_s3 is None:
                        ev = ev_tiles[g % 4]
                        nc.scalar.copy(ev[:, 0:D], ps[:])
                        nc.sync.dma_start(
                            dst_shard.opt()[g * GSZ:(g + 1) * GSZ, :], ev[:])
                    else:
                        nc.scalar.copy(dst_s3[:, g, :], ps[:])
                if last_pool is not None:
                    anchor[0] = last_pool

            def run_l3(t, sv_in, idx_in, src_tab, qsize, glist):
                """Source-sharded L3 pass: ragged per-group chunk counts,
                super-gathers over SG consecutive groups from the LOCAL s2
                shard, contrib-table destination rows.  Generator: yields
                after each super-group so it can interleave under another
                pass."""
                C, qoff, totC = t["C"], t["qoff"], t["totC"]
                nG = len(glist)
                SG = 8
                prev_anchor, last_pool = anchor[0], None
                for si in range(0, nG, SG):
                    if si > 0:
                        yield
                    glo, ghi = si, min(nG, si + SG)
                    clo, chi = int(qoff[glo]), int(qoff[ghi])
                    cq = chi - clo
                    svt = svp.tile([CH, 2 * cq], mybir.dt.float32,
                                   name="sv3", tag="sv")
                    nc.sync.dma_start(svt[:],
                                      sv_in.ap()[:, 2 * clo:2 * chi])
                    ixt = ixp.tile([128, cq * 8], i16, name="ixt3",
                                   tag="ixt")
                    nc.sync.dma_start(ixt[:],
                                      idx_in.ap()[:, clo * 8:chi * 8])
                    gt = gtp.tile([CH, cq, DP], bf16, name="gt3", tag="gtg")
                    gi = nc.gpsimd.dma_gather(
                        gt[:], src_tab.opt()[0:qsize], ixt[:],
                        cq * CH, cq * CH, DP, single_packet=False)
                    if si < 4 * SG and prev_anchor is not None:
                        add_dep_helper(gi.ins, prev_anchor.ins,
                                       reason="pool queue order")
                    last_pool = gi
                    evb = evp.tile([GSZ, SG, DP], bf16, name="evb",
                                   tag="evb")
                    nc.vector.memset(evb[:], 0.0)
                    for g in range(glo, ghi):
                        ps = psp.tile([GSZ, D], mybir.dt.float32, name="ps",
                                      tag="ps", space="PSUM")
                        cg = int(C[g])
                        for k in range(cg):
                            cx = int(qoff[g]) + k - clo
                            oh = ohp.tile([CH, GSZ], bf16, name="oh",
                                          tag="oh")
                            nc.vector.tensor_scalar(
                                out=oh[:], in0=iota_t[:],
                                scalar1=svt[:, 2 * cx:2 * cx + 1],
                                scalar2=svt[:, 2 * cx + 1:2 * cx + 2],
                                op0=EQ, op1=MUL)
                            nc.tensor.matmul(ps[:], lhsT=oh[:],
                                             rhs=gt[:, cx, 0:D],
                                             start=(k == 0),
                                             stop=(k == cg - 1))
                        nc.scalar.copy(evb[:, g - glo, 0:D], ps[:])
                    # super-group rows are contiguous in contrib
                    nc.sync.dma_start(
                        contrib.opt()[glist[glo]:glist[glo] + SG * GSZ, :]
                        .rearrange("(j p) d -> p j d", p=GSZ), evb[:])
                if last_pool is not None:
                    anchor[0] = last_pool

            def ag(shard, full):
                nc.gpsimd.collective_compute(
                    "AllGather", mybir.AluOpType.bypass, replica_groups=reps,
                    ins=[shard.opt()], outs=[full.opt()])

            # AG(s1_i) overlaps iu-L1; AG(s1_u) overlaps iu-L2 (which
            # consumes s1_i).  L2 writes stay shard-local; L3 is
            # source-sharded and gathers from the LOCAL s2 shards, writing
            # batch contributions that a single ReduceScatter sums.
            run_pass(lays["ui"], None, 0, 0, sh["s1_i"], None)
            ag(sh["s1_i"], fl["s1_i"])
            run_pass(lays["iu"], None, 0, 0, sh["s1_u"], None)
            ag(sh["s1_u"], fl["s1_u"])
            run_pass(lays["iu"], fl["s1_i"], N_IT, NQ_IT, sh["s2_u"], None)
            g3a = run_l3(lays["3a"], lays["3a"]["sv"], lays["3a"]["idx"],
                         sh["s2_u"], GU_C * GSZ, p["glist_a"])
            run_pass(lays["ui"], fl["s1_u"], N_US, NQ_US, sh["s2_i"], None,
                     hook=g3a)
            for _ in g3a:
                pass
            for _ in run_l3(lays["3b"], lays["3b"]["sv"], lays["3b"]["idx"],
                            sh["s2_i"], GI_C * GSZ, p["glist_b"]):
                pass
            nc.gpsimd.collective_compute(
                "ReduceScatter", mybir.AluOpType.add, replica_groups=reps,
                ins=[contrib.opt()], outs=[rs3o.opt()])

            qv = {k: fl[k].opt().rearrange("(n r) d -> n (r d)", r=4)
                  for k in fl}

            # batched readout quad gathers for the layer-1 terms
            gq = {}
            for nm, idxr in (("s1_u", idxr_u), ("s1_i", idxr_i)):
                ixr = roqp.tile([128, BPC // 16], i16, name=f"ixr_{nm}")
                nc.sync.dma_start(ixr[:], idxr.ap())
                gq[nm] = roqp.tile([RG, NRG, 4 * DP], bf16, name=f"gq_{nm}")
                gi = nc.gpsimd.dma_gather(gq[nm][:], qv[nm], ixr[:], BPC,
                                          BPC, 4 * DP, single_packet=False)
                if anchor[0] is not None:
                    add_dep_helper(gi.ins, anchor[0].ins,
                                   reason="pool queue order")

            def side(rg, maskt, g1, e0t, part_s2, part_s3):
                mk = rop.tile([RG, 4 * DP], bf16, name="mk", tag="mk")
                nc.sync.dma_start(mk[:], maskt.ap()[rg])
                e0 = rop.tile([RG, D], mybir.dt.float32, name="e0", tag="e0")
                nc.sync.dma_start(e0[:], e0t.ap()[rg])
                acc = rop.tile([RG, D], mybir.dt.float32, name="acc",
                               tag="acc")
                r2 = rop.tile([RG, DP], bf16, name="r2", tag="r2")
                nc.sync.dma_start(
                    r2[:], rs3o.opt()[part_s2 * BPC + rg * RG:
                                      part_s2 * BPC + (rg + 1) * RG, :])
                r3 = rop.tile([RG, DP], bf16, name="r3", tag="r3")
                nc.sync.dma_start(
                    r3[:], rs3o.opt()[part_s3 * BPC + rg * RG:
                                      part_s3 * BPC + (rg + 1) * RG, :])
                nc.vector.tensor_add(out=acc[:], in0=e0[:], in1=r2[:, 0:D])
                nc.vector.tensor_add(out=acc[:], in0=acc[:], in1=r3[:, 0:D])
                sel = rop.tile([RG, 4 * DP], mybir.dt.float32,
                               name="sel", tag="sel")
                nc.vector.tensor_mul(out=sel[:], in0=g1[:, rg, :],
                                     in1=mk[:])
                red = rop.tile([RG, D], mybir.dt.float32, name="red",
                               tag="red")
                nc.vector.reduce_sum(
                    red[:],
                    sel[:].rearrange("p (r d) -> p d r", r=4)[:, 0:D, :],
                    axis=mybir.AxisListType.X)
                nc.vector.tensor_add(out=acc[:], in0=acc[:], in1=red[:])
                return acc

            yv = y_out.ap().rearrange("(g p) -> g p", p=RG)
            for rg in range(NRG):
                # contrib parts per core block: [ci3 | cu2 | cu3 | ci2]
                su = side(rg, mask_u, gq["s1_u"], e0su, 1, 2)
                si = side(rg, mask_i, gq["s1_i"], e0si, 3, 0)
                pr = rop.tile([RG, D], mybir.dt.float32, name="pr", tag="pr")
                nc.vector.tensor_mul(out=pr[:], in0=su[:], in1=si[:])
                nc.vector.tensor_scalar_mul(out=pr[:], in0=pr[:],
                                            scalar1=1.0 / 16.0)
                yc = rop.tile([RG, 1], mybir.dt.float32, name="yc", tag="yc")
                nc.vector.reduce_sum(yc[:], pr[:], axis=mybir.AxisListType.X)
                nc.sync.dma_start(yv[rg], yc[:, 0])

    nc.compile()
    return nc


_CACHE = {}
_TRACE = False
_TRACE_DIR = None
_LAST_RES = None


def _schedule_key(p):
    import hashlib
    h = hashlib.sha1()
    for k in ("ui", "iu"):
        h.update(p[k]["C"].tobytes())
        h.update(np.int64(p[k]["sumC"]).tobytes())
    for k in ("l3a", "l3b"):
        h.update(p[k]["C"].tobytes())
        h.update(np.int64(p[k]["totC"]).tobytes())
    return h.hexdigest()


def kernel(user_emb, item_emb, edge_vals, edge_u, edge_i, users, items):
    global _LAST_RES
    from concourse.bass_utils import run_bass_kernel_spmd

    user_emb = np.asarray(user_emb, np.float32)
    item_emb = np.asarray(item_emb, np.float32)
    edge_vals = np.asarray(edge_vals, np.float32)
    edge_u = np.asarray(edge_u, np.int64)
    edge_i = np.asarray(edge_i, np.int64)
    users = np.asarray(users, np.int64)
    items = np.asarray(items, np.int64)

    p = _make_plan(user_emb, item_emb, edge_vals, edge_u, edge_i, users,
                   items)
    maps = _build_device_arrays(p)
    key = _schedule_key(p)
    if _CACHE.get("key") != key:
        _CACHE["nc"] = _build_bass(p)
        _CACHE["key"] = key
    nc = _CACHE["nc"]
    res = run_bass_kernel_spmd(nc, maps, core_ids=list(range(NCORES)),
                               trace=_TRACE, tmpdir=_TRACE_DIR)
    _LAST_RES = res
    y = np.concatenate([res.results[c]["y"] for c in range(NCORES)])
    return y.astype(np.float32)

